# revision 38
# baseline (speedup 1.0000x reference)
"""BoundaryAttentionModule Trainium2 kernel — centered moment expansion, fp8 DR.

Shapes (hardcoded): b=4, c=256, h=w=64 (HW=4096), mid=64, out_ch=256.
8 cores: core = (batch bi = core//2, key-half kh = core%2); each core
handles its 2048 keys against all 4096 queries j.

Math: E^T[k,j] = t_k*A_S[j] + B_S[j] within ReLU-region S of the scalar
boundary value t_k.  Expansion is CENTERED per region: with region
center t_S and half-width h_S, U[k,j] = exp(B'_S[j]) * exp(d A'_S[j])
where B' = B + t_S A, A' = h_S A, d = (t_k - t_S)/h_S in [-1,1].  The
host splits wide regions (64 region slots) so |d A'| is tiny and TWO
Taylor orders suffice: U ~ W0 + d*W1, W0 = exp(B'), W1 = W0*A'.
Host folds M = key_w2^T @ query_w into CA/CB: A'/B' come straight from
u via one fp8 DoubleRow matmul each (contraction c=256), no G2.

W is held as two per-j-half tiles Wj[h] [128=(n,S), 2048] (separate
tiles + separate accumulators give the Tile scheduler fine-grained
deps): rows 0:64 = W0 = exp(B') (ACT exp from psum, sigma0 via accum),
rows 64:128 = W1 = W0*A' (one scalar_tensor_tensor per half on DVE,
sigma1 via accum).  Per-half s matmuls start as soon as that half's
chain ends.  vT (fp8 DR, keys = j 0:2048) runs in the chain window;
pws = PSCALE*pw/s in bf16 (DVE 4x); moment + P in bf16; output fp8
written as two contiguous 512KB DMAs (host divides PSCALE via gamma).

HW notes baked in here: fp8 DoubleRow gives 2x contraction depth but
no column-rate gain; DVE fp8-dst elementwise ops are ~16x slow (keep
bf16); GPSIMD cannot touch PSUM; DMA throughput needs >=4KB-contiguous
runs on the HW-DGE queues (sync/scalar start fast, gpsimd aggregates
but starts ~4us late); the PE needs ~3us of continuous work to reach
2.4GHz (warm-up matmuls bridge the input-DMA window).
"""

import numpy as np

B, C, HW = 4, 256, 4096
KH = HW // 2          # 2048 keys per core
NKT = KH // 128       # 16 key tiles
RP = 64               # region slots
NORD = 2              # Taylor orders 0..1 (centered)
BASIS = NORD * RP     # 128
PSCALE = 128.0        # pws scale folded out on host via gamma

TRACE = False
TRACE_CORES = None
LAST_RESULTS = None

_BUILT = None


def _build():
    import concourse.bass as bass
    import concourse.tile as tile
    from concourse import bacc, mybir

    f32 = mybir.dt.float32
    bf16 = mybir.dt.bfloat16
    f8 = mybir.dt.float8e4
    AF = mybir.ActivationFunctionType
    AX = mybir.AxisListType
    ALU = mybir.AluOpType
    DR = mybir.MatmulPerfMode.DoubleRow

    nc = bacc.Bacc(
        "TRN2",
        target_bir_lowering=False,
        debug=False,
        enable_asserts=False,
        num_devices=8,
    )

    u8_in = nc.dram_tensor("u8_in", [128, 2, HW], f8, kind="ExternalInput").ap()
    cab8_in = nc.dram_tensor("cab8_in", [128, 2, 2 * RP], f8, kind="ExternalInput").ap()
    vw8_in = nc.dram_tensor("vw8_in", [128, 2, C], f8, kind="ExternalInput").ap()
    pwsb_in = nc.dram_tensor("pwsb_in", [128, NKT * BASIS], bf16, kind="ExternalInput").ap()
    pwt_in = nc.dram_tensor("pwt_in", [BASIS, KH], bf16, kind="ExternalInput").ap()
    p_out = nc.dram_tensor("p_out", [2, 128, HW], f8, kind="ExternalOutput").ap()

    with tile.TileContext(nc) as tc:
        with (
            tc.tile_pool(name="sb", bufs=1) as sb,
            tc.tile_pool(name="ab", bufs=4, space="PSUM") as abp,
            tc.tile_pool(name="wm", bufs=1, space="PSUM") as wmp,
            tc.tile_pool(name="vt", bufs=2, space="PSUM") as vtp,
            tc.tile_pool(name="pin", bufs=1, space="PSUM") as pinp,
        ):
            # ---- SBUF tiles ----
            u8 = sb.tile([128, 2, HW], f8, tag="u8", name="u8")
            cab8 = sb.tile([128, 2, 2 * RP], f8, tag="cab8", name="cab8")
            vw8 = sb.tile([128, 2, C], f8, tag="vw8", name="vw8")
            pwsb = sb.tile([128, NKT * BASIS], bf16, tag="pwsb", name="pwsb")
            pwsB = sb.tile([128, NKT * BASIS], bf16, tag="pwsB", name="pwsB")
            pwt = sb.tile([BASIS, KH], bf16, tag="pwt", name="pwt")
            Af0 = sb.tile([64, KH], bf16, tag="Af0", name="Af0")
            Af1 = sb.tile([64, KH], bf16, tag="Af1", name="Af1")
            Wj0 = sb.tile([128, KH], bf16, tag="Wj0", name="Wj0")
            Wj1 = sb.tile([128, KH], bf16, tag="Wj1", name="Wj1")
            AfH = (Af0, Af1)
            WjH = (Wj0, Wj1)
            vtb = sb.tile([128, NKT * C], bf16, tag="vtb", name="vtb")
            saccE = sb.tile([64, 8], f32, tag="saccE", name="saccE")
            saccC = sb.tile([64, 2], f32, tag="saccC", name="saccC")
            sigf = sb.tile([64, 2], f32, tag="sigf", name="sigf")
            sigb = sb.tile([128, 2], bf16, tag="sigb", name="sigb")
            rinv = sb.tile([128, NKT], f32, tag="rinv", name="rinv")
            mo0 = sb.tile([128, C], bf16, tag="mo0", name="mo0")
            po = sb.tile([128, 2 * HW], f8, tag="po", name="po")
            scr = sb.tile([128, 512], bf16, tag="scr", name="scr")
            nc.vector.memset(scr[:], 0.0)

            spin = pinp.tile([128, 512], f32, tag="spin", name="spin")
            s_ps = spin[:, 0 : 2 * NKT]
            mo_ps = spin[:, 256 : 256 + C]
            ssum = sb.tile([128, NKT], f32, tag="ssum", name="ssum")

            # ---- input DMAs ----
            # Only the HW-DGE queues (sync/scalar) start promptly; gpsimd's
            # SW-DGE adds ~4us. c-half u slices are 4KB runs -> fast packets.
            nc.gpsimd.dma_start(cab8[:], cab8_in[:, :, :])
            nc.gpsimd.dma_start(vw8[:], vw8_in[:, :, :])
            nc.sync.dma_start(u8[:, 0:1, 0:1024], u8_in[:, 0:1, 0:1024])
            nc.scalar.dma_start(u8[:, 1:2, 0:1024], u8_in[:, 1:2, 0:1024])
            nc.sync.dma_start(u8[:, 0:1, 1024:KH], u8_in[:, 0:1, 1024:KH])
            nc.scalar.dma_start(u8[:, 1:2, 1024:KH], u8_in[:, 1:2, 1024:KH])
            nc.scalar.dma_start(u8[:, 0:1, KH:HW], u8_in[:, 0:1, KH:HW])
            nc.sync.dma_start(u8[:, 1:2, KH:HW], u8_in[:, 1:2, KH:HW])
            nc.gpsimd.dma_start(pwt[:], pwt_in[:, :])
            nc.gpsimd.dma_start(pwsb[:], pwsb_in[:, :])

            # ---- PE warm-up while inputs stream ----
            def warm(i, n=1):
                for k in range(n):
                    pwm = wmp.tile([128, 512], f32, tag="wm", name=f"warm{i}_{k}")
                    nc.tensor.matmul(
                        pwm[:], scr[:, 0:128], scr[:, 0:512], start=True, stop=True
                    )

            warm("pre", 15)

            # ---- A|B matmul (one fp8 DR mm: out rows 0:64=A, 64:128=B),
            # exp from rows 64:128, Af copy from rows 0:64 ----
            def ab_chunk(ci, k):
                j0 = 512 * ci
                h, jj = ci // 4, 512 * (ci % 4)
                ptile = abp.tile([128, 512], f32, tag="pa", name=f"pab{ci}")
                nc.tensor.matmul(
                    ptile[:, 0:512], cab8[:, :, 0:128],
                    u8[:, :, j0 : j0 + 512],
                    start=True, stop=True, perf_mode=DR,
                )
                nc.scalar.activation(
                    WjH[h][0:64, jj : jj + 512], ptile[64:128, 0:512], AF.Exp,
                    accum_out=saccE[0:64, k : k + 1],
                )
                if k in (5, 7):
                    nc.scalar.copy(AfH[h][0:64, jj : jj + 512], ptile[0:64, 0:512])
                else:
                    nc.vector.tensor_copy(AfH[h][0:64, jj : jj + 512], ptile[0:64, 0:512])
                return k

            def vt_pair(kt, do_copy=True):
                pv = vtp.tile([128, 2 * C], f32, tag="pv", name=f"pv{kt}")
                for q in range(2):
                    nc.tensor.matmul(
                        pv[:, q * C : (q + 1) * C],
                        u8[:, :, (kt + q) * 128 : (kt + q + 1) * 128],
                        vw8[:, :, :],
                        start=True, stop=True, perf_mode=DR,
                    )
                if do_copy:
                    nc.scalar.copy(vtb[:, kt * C : (kt + 2) * C], pv[:, 0 : 2 * C])
                return pv

            # chunk order: j-halves interleaved so keys (cols 0:2048) and
            # chain inputs both complete early.  Chain half h's DVE work is
            # emitted as soon as its 4 chunks are in; the s matmuls are
            # emitted after ALL chunks so they never block chunk matmuls in
            # the in-order PE stream.
            def chain_stt(half):
                Wj = WjH[half]
                nc.vector.scalar_tensor_tensor(
                    Wj[64:128, :], Wj[0:64, :], 1.0,
                    AfH[half][0:64, :],
                    op0=ALU.mult, op1=ALU.mult,
                    accum_out=saccC[0:64, half : half + 1],
                )
                nc.vector.reduce_sum(
                    sigf[0:64, half : half + 1],
                    saccE[0:64, 4 * half : 4 * half + 4], axis=AX.X,
                )
                nc.vector.tensor_copy(
                    sigb[0:64, half : half + 1], sigf[0:64, half : half + 1]
                )
                nc.vector.tensor_copy(
                    sigb[64:128, half : half + 1], saccC[0:64, half : half + 1]
                )

            def s_mms(half):
                for kt in range(NKT):
                    nc.tensor.matmul(
                        s_ps[:, half * NKT + kt : half * NKT + kt + 1],
                        pwt[:, kt * 128 : (kt + 1) * 128],
                        sigb[:, half : half + 1],
                        start=True, stop=True,
                    )
                if half == 0:
                    nc.vector.tensor_copy(ssum[:], s_ps[:, 0:NKT])

            CHUNKS = (0, 1, 2, 3, 4, 5, 6, 7)
            for k, ci in enumerate(CHUNKS):
                ab_chunk(ci, k)
                if k == 3:
                    chain_stt(0)
            chain_stt(1)
            # vT runs in the chain window: PE + ACT are otherwise idle here.
            # The last two pair-copies go to DVE, interleaved into the pws
            # stream below, so ACT finishes in time for the moment phase.
            vt_late = {}
            for kt in range(0, NKT, 2):
                if kt < 12:
                    vt_pair(kt)
                else:
                    vt_late[kt] = vt_pair(kt, do_copy=False)
            s_mms(0)
            s_mms(1)
            nc.vector.tensor_tensor(
                ssum[:], ssum[:], s_ps[:, NKT : 2 * NKT], op=ALU.add
            )
            nc.vector.reciprocal(rinv[:], ssum[:])

            # ---- pws = pwsb * rinv (bf16, DVE 4x), then moment (bf16) ----
            for kt in range(NKT):
                if kt in vt_late:
                    nc.vector.tensor_copy(
                        vtb[:, kt * C : (kt + 2) * C], vt_late[kt][:, 0 : 2 * C]
                    )
                nc.vector.tensor_scalar(
                    pwsB[:, kt * BASIS : (kt + 1) * BASIS],
                    pwsb[:, kt * BASIS : (kt + 1) * BASIS],
                    rinv[:, kt : kt + 1], None, op0=ALU.mult,
                )
                nc.tensor.matmul(
                    mo_ps[:],
                    pwsB[:, kt * BASIS : (kt + 1) * BASIS],
                    vtb[:, kt * C : (kt + 1) * C],
                    start=(kt == 0), stop=(kt == NKT - 1),
                )
            nc.scalar.copy(mo0[:], mo_ps[:])

            # ---- P = Mo^T @ W -> fp8 out ----
            # psum ring alternates pa/pb tags (4-deep) so P matmuls never
            # stall on the copy drain; one contiguous 512KB DMA per c-block.
            for ct in range(2):
                for jg in range(8):
                    pp = abp.tile([128, 512], f32, tag="pa", name=f"pp{ct}_{jg}")
                    nc.tensor.matmul(
                        pp[:],
                        mo0[:, ct * 128 : (ct + 1) * 128],
                        WjH[jg // 4][:, (jg % 4) * 512 : (jg % 4 + 1) * 512],
                        start=True, stop=True,
                    )
                    dst = po[:, ct * HW + jg * 512 : ct * HW + (jg + 1) * 512]
                    if jg % 2 == 0:
                        nc.scalar.copy(dst, pp[:])
                    else:
                        nc.vector.tensor_copy(dst, pp[:])
                if ct == 0:
                    nc.gpsimd.dma_start(p_out[0:1, :, :], po[:, 0:HW])
                else:
                    nc.sync.dma_start(p_out[1:2, 0:48, :], po[0:48, HW : 2 * HW])
                    nc.scalar.dma_start(p_out[1:2, 48:96, :], po[48:96, HW : 2 * HW])
                    nc.gpsimd.dma_start(p_out[1:2, 96:128, :], po[96:128, HW : 2 * HW])

    nc.compile()
    return nc


def _get_built():
    global _BUILT
    if _BUILT is None:
        _BUILT = _build()
    return _BUILT


def _regions(kw1f, beta, t):
    """Region edges: ReLU breakpoints inside t-range, merged to <= RP-1,
    then wide regions split so max |t - center| shrinks (all slots used)."""
    tmin, tmax = t.min(), t.max()
    bp = -beta / np.where(np.abs(kw1f) < 1e-30, 1e-30, kw1f)
    inr = np.sort(bp[(bp > tmin) & (bp < tmax)])
    while len(inr) > RP - 1:
        gaps = np.diff(np.concatenate([[tmin], inr, [tmax]]))
        i = int(np.argmin(gaps[:-1] + gaps[1:]))
        inr = np.delete(inr, i)
    edges = list(inr)
    while len(edges) < RP - 1:
        full = np.concatenate([[tmin - 1e-9], edges, [tmax + 1e-9]])
        bi, bm, bsplit = -1, -1.0, None
        for i in range(len(full) - 1):
            selm = t[(t > full[i]) & (t <= full[i + 1])]
            if len(selm) < 2:
                continue
            c = 0.5 * (selm.min() + selm.max())
            m = np.abs(selm - c).max()
            if m > bm:
                bm, bi, bsplit = m, i, float(np.median(selm))
        if bi < 0:
            break
        edges.append(bsplit)
        edges.sort()
    return np.array(edges)


def _host_prep(boundary_map, uncertainty_map, key_w1, bn_scale, bn_bias,
               bn_mean, bn_var, key_w2, query_w, value_w):
    import ml_dtypes

    bf = ml_dtypes.bfloat16
    f8 = ml_dtypes.float8_e4m3
    b, c, h, w = uncertainty_map.shape
    H0 = boundary_map.shape[2]
    idx = (np.arange(h) * H0) // h
    bm = boundary_map[:, 0][:, idx][:, :, idx].reshape(b, h * w).astype(np.float64)

    inv = bn_scale.astype(np.float64) / np.sqrt(bn_var.astype(np.float64) + 1e-5)
    beta = bn_bias.astype(np.float64) - bn_mean.astype(np.float64) * inv
    kw1f = key_w1[:, 0].astype(np.float64) * inv
    m_t = key_w2.T.astype(np.float64) @ query_w.astype(np.float64)   # [64, 256]
    vw_t = np.ascontiguousarray(value_w.T.astype(np.float64))        # [256, 256]
    vw8 = vw_t.reshape(2, 128, C).transpose(1, 0, 2)                 # [128,2,256]

    in_maps = []
    for core in range(8):
        bi, kh = core // 2, core % 2
        t_full = bm[bi]
        tk = t_full[kh * KH : (kh + 1) * KH]
        u = uncertainty_map[bi].reshape(c, h * w).astype(np.float64)
        u = np.roll(u, -kh * KH, axis=1)
        u8 = u.reshape(2, 128, HW).transpose(1, 0, 2)                # [128,2,HW]

        edges = _regions(kw1f, beta, tk)
        R = len(edges) + 1
        reg = np.searchsorted(edges, tk)                             # [KH]
        lo = np.concatenate([[tk.min() - 1e-9], edges])
        hi = np.concatenate([edges, [tk.max() + 1e-9]])
        relu_mid = 0.5 * (lo + hi)
        masks = (kw1f[None, :] * relu_mid[:, None] + beta[None, :]) > 0  # [R,64]
        tc = np.zeros(R)
        hh = np.ones(R)
        for r_ in range(R):
            selk = tk[reg == r_]
            if len(selk):
                tc[r_] = 0.5 * (selk.min() + selk.max())
                hh[r_] = max(np.abs(selk - tc[r_]).max(), 1e-6)
        ca = masks * kw1f[None, :]                                   # [R,64]
        cb = masks * beta[None, :]
        camT = ((hh[:, None] * ca) @ m_t).T                          # [256, R]
        cbmT = ((cb + tc[:, None] * ca) @ m_t).T                     # [256, R]
        cabf = np.zeros((256, 2 * RP))
        cabf[:, 0:R] = camT
        cabf[:, RP : RP + R] = cbmT
        cab8 = cabf.reshape(2, 128, 2 * RP).transpose(1, 0, 2)       # [128,2,128]

        dlt = (tk - tc[reg]) / hh[reg]                               # [KH]
        pw = np.zeros((KH, BASIS))
        krange = np.arange(KH)
        pw[krange, reg] = 1.0
        pw[krange, RP + reg] = dlt
        # pwsb [128 kw, kt*BASIS+bc] = PSCALE * pw[kt*128+kw, bc]
        # (PSCALE folded back out on host via gamma)
        pwsb = (PSCALE * pw).reshape(NKT, 128, BASIS).transpose(1, 0, 2).reshape(
            128, NKT * BASIS)
        in_maps.append({
            "u8_in": np.ascontiguousarray(u8).astype(f8),
            "cab8_in": np.ascontiguousarray(cab8).astype(f8),
            "vw8_in": np.ascontiguousarray(vw8).astype(f8),
            "pwsb_in": np.ascontiguousarray(pwsb).astype(bf),
            "pwt_in": np.ascontiguousarray(pw.T).astype(bf),
        })
    return in_maps


def kernel(boundary_map, uncertainty_map, key_w1, bn_scale, bn_bias,
           bn_mean, bn_var, key_w2, query_w, value_w, gamma):
    global LAST_RESULTS
    from concourse.bass_utils import run_bass_kernel_spmd

    nc = _get_built()
    in_maps = _host_prep(
        np.asarray(boundary_map), np.asarray(uncertainty_map), np.asarray(key_w1),
        np.asarray(bn_scale), np.asarray(bn_bias), np.asarray(bn_mean),
        np.asarray(bn_var), np.asarray(key_w2), np.asarray(query_w),
        np.asarray(value_w),
    )
    kwargs = {}
    if TRACE:
        kwargs["trace"] = True
        if TRACE_CORES is not None:
            kwargs["trace_cores"] = TRACE_CORES
    res = run_bass_kernel_spmd(nc, in_maps, core_ids=list(range(8)), **kwargs)
    LAST_RESULTS = res

    b, c, h, w = uncertainty_map.shape
    g = np.float64(np.asarray(gamma).reshape(-1)[0]) / PSCALE
    out = np.empty((b, c, h * w), np.float32)
    um = np.asarray(uncertainty_map)
    for bi in range(b):
        P = (res.results[2 * bi]["p_out"].astype(np.float32).reshape(C, HW)
             + np.roll(res.results[2 * bi + 1]["p_out"].astype(np.float32).reshape(C, HW),
                       KH, axis=1))
        out[bi] = g * P + um[bi].reshape(c, h * w)
    return out.reshape(b, c, h, w)


# revision 39
# speedup vs baseline: 1.0076x; 1.0076x over previous
"""BoundaryAttentionModule Trainium2 kernel — centered moment expansion, fp8 DR.

Shapes (hardcoded): b=4, c=256, h=w=64 (HW=4096), mid=64, out_ch=256.
8 cores: core = (batch bi = core//2, key-half kh = core%2); each core
handles its 2048 keys against all 4096 queries j.

Math: E^T[k,j] = t_k*A_S[j] + B_S[j] within ReLU-region S of the scalar
boundary value t_k.  Expansion is CENTERED per region: with region
center t_S and half-width h_S, U[k,j] = exp(B'_S[j]) * exp(d A'_S[j])
where B' = B + t_S A, A' = h_S A, d = (t_k - t_S)/h_S in [-1,1].  The
host splits wide regions (64 region slots) so |d A'| is tiny and TWO
Taylor orders suffice: U ~ W0 + d*W1, W0 = exp(B'), W1 = W0*A'.
Host folds M = key_w2^T @ query_w into CA/CB: A'/B' come straight from
u via one fp8 DoubleRow matmul each (contraction c=256), no G2.

W is held as two per-j-half tiles Wj[h] [128=(n,S), 2048] (separate
tiles + separate accumulators give the Tile scheduler fine-grained
deps): rows 0:64 = W0 = exp(B') (ACT exp from psum, sigma0 via accum),
rows 64:128 = W1 = W0*A' (one scalar_tensor_tensor per half on DVE,
sigma1 via accum).  Per-half s matmuls start as soon as that half's
chain ends.  vT (fp8 DR, keys = j 0:2048) runs in the chain window;
pws = PSCALE*pw/s in bf16 (DVE 4x); moment + P in bf16; output fp8
written as two contiguous 512KB DMAs (host divides PSCALE via gamma).

HW notes baked in here: fp8 DoubleRow gives 2x contraction depth but
no column-rate gain; DVE fp8-dst elementwise ops are ~16x slow (keep
bf16); GPSIMD cannot touch PSUM; DMA throughput needs >=4KB-contiguous
runs on the HW-DGE queues (sync/scalar start fast, gpsimd aggregates
but starts ~4us late); the PE needs ~3us of continuous work to reach
2.4GHz (warm-up matmuls bridge the input-DMA window).
"""

import numpy as np

B, C, HW = 4, 256, 4096
KH = HW // 2          # 2048 keys per core
NKT = KH // 128       # 16 key tiles
RP = 64               # region slots
NORD = 2              # Taylor orders 0..1 (centered)
BASIS = NORD * RP     # 128
PSCALE = 128.0        # pws scale folded out on host via gamma

TRACE = False
TRACE_CORES = None
LAST_RESULTS = None

_BUILT = None


def _build():
    import concourse.bass as bass
    import concourse.tile as tile
    from concourse import bacc, mybir

    f32 = mybir.dt.float32
    bf16 = mybir.dt.bfloat16
    f8 = mybir.dt.float8e4
    AF = mybir.ActivationFunctionType
    AX = mybir.AxisListType
    ALU = mybir.AluOpType
    DR = mybir.MatmulPerfMode.DoubleRow

    nc = bacc.Bacc(
        "TRN2",
        target_bir_lowering=False,
        debug=False,
        enable_asserts=False,
        num_devices=8,
    )

    u8_in = nc.dram_tensor("u8_in", [128, 2, HW], f8, kind="ExternalInput").ap()
    cab8_in = nc.dram_tensor("cab8_in", [128, 2, 2 * RP], f8, kind="ExternalInput").ap()
    vw8_in = nc.dram_tensor("vw8_in", [128, 2, C], f8, kind="ExternalInput").ap()
    pwsb_in = nc.dram_tensor("pwsb_in", [128, NKT * BASIS], bf16, kind="ExternalInput").ap()
    pwt_in = nc.dram_tensor("pwt_in", [BASIS, KH], bf16, kind="ExternalInput").ap()
    p_out = nc.dram_tensor("p_out", [2, 128, HW], f8, kind="ExternalOutput").ap()

    with tile.TileContext(nc) as tc:
        with (
            tc.tile_pool(name="sb", bufs=1) as sb,
            tc.tile_pool(name="ab", bufs=4, space="PSUM") as abp,
            tc.tile_pool(name="wm", bufs=1, space="PSUM") as wmp,
            tc.tile_pool(name="vt", bufs=2, space="PSUM") as vtp,
            tc.tile_pool(name="pin", bufs=1, space="PSUM") as pinp,
        ):
            # ---- SBUF tiles ----
            u8 = sb.tile([128, 2, HW], f8, tag="u8", name="u8")
            cab8 = sb.tile([128, 2, 2 * RP], f8, tag="cab8", name="cab8")
            vw8 = sb.tile([128, 2, C], f8, tag="vw8", name="vw8")
            pwsb = sb.tile([128, NKT * BASIS], bf16, tag="pwsb", name="pwsb")
            pwsB = sb.tile([128, NKT * BASIS], bf16, tag="pwsB", name="pwsB")
            pwt = sb.tile([BASIS, KH], bf16, tag="pwt", name="pwt")
            Af0 = sb.tile([64, KH], bf16, tag="Af0", name="Af0")
            Af1 = sb.tile([64, KH], bf16, tag="Af1", name="Af1")
            Wj0 = sb.tile([128, KH], bf16, tag="Wj0", name="Wj0")
            Wj1 = sb.tile([128, KH], bf16, tag="Wj1", name="Wj1")
            AfH = (Af0, Af1)
            WjH = (Wj0, Wj1)
            vtb = sb.tile([128, NKT * C], bf16, tag="vtb", name="vtb")
            saccE = sb.tile([64, 8], f32, tag="saccE", name="saccE")
            saccC = sb.tile([64, 2], f32, tag="saccC", name="saccC")
            sigf = sb.tile([64, 2], f32, tag="sigf", name="sigf")
            sigb = sb.tile([128, 2], bf16, tag="sigb", name="sigb")
            rinv = sb.tile([128, NKT], f32, tag="rinv", name="rinv")
            mo0 = sb.tile([128, C], bf16, tag="mo0", name="mo0")
            po = sb.tile([128, 2 * HW], f8, tag="po", name="po")
            scr = sb.tile([128, 512], bf16, tag="scr", name="scr")
            nc.vector.memset(scr[:], 0.0)

            spin = pinp.tile([128, 512], f32, tag="spin", name="spin")
            s_ps = spin[:, 0 : 2 * NKT]
            mo_ps = spin[:, 256 : 256 + C]
            ssum = sb.tile([128, NKT], f32, tag="ssum", name="ssum")

            # ---- input DMAs ----
            # Only the HW-DGE queues (sync/scalar) start promptly; gpsimd's
            # SW-DGE adds ~4us. c-half u slices are 4KB runs -> fast packets.
            nc.gpsimd.dma_start(cab8[:], cab8_in[:, :, :])
            nc.gpsimd.dma_start(vw8[:], vw8_in[:, :, :])
            nc.sync.dma_start(u8[:, 0:1, 0:1024], u8_in[:, 0:1, 0:1024])
            nc.scalar.dma_start(u8[:, 1:2, 0:1024], u8_in[:, 1:2, 0:1024])
            nc.sync.dma_start(u8[:, 0:1, 1024:KH], u8_in[:, 0:1, 1024:KH])
            nc.scalar.dma_start(u8[:, 1:2, 1024:KH], u8_in[:, 1:2, 1024:KH])
            nc.gpsimd.dma_start(u8[:, 0:1, KH:HW], u8_in[:, 0:1, KH:HW])
            nc.sync.dma_start(u8[:, 1:2, KH:HW], u8_in[:, 1:2, KH:HW])
            nc.scalar.dma_start(pwt[:], pwt_in[:, :])
            nc.scalar.dma_start(pwsb[:], pwsb_in[:, :])

            # ---- PE warm-up while inputs stream ----
            def warm(i, n=1):
                for k in range(n):
                    pwm = wmp.tile([128, 512], f32, tag="wm", name=f"warm{i}_{k}")
                    nc.tensor.matmul(
                        pwm[:], scr[:, 0:128], scr[:, 0:512], start=True, stop=True
                    )

            warm("pre", 15)

            # ---- A|B matmul (one fp8 DR mm: out rows 0:64=A, 64:128=B),
            # exp from rows 64:128, Af copy from rows 0:64 ----
            def ab_chunk(ci, k):
                j0 = 512 * ci
                h, jj = ci // 4, 512 * (ci % 4)
                ptile = abp.tile([128, 512], f32, tag="pa", name=f"pab{ci}")
                nc.tensor.matmul(
                    ptile[:, 0:512], cab8[:, :, 0:128],
                    u8[:, :, j0 : j0 + 512],
                    start=True, stop=True, perf_mode=DR,
                )
                nc.scalar.activation(
                    WjH[h][0:64, jj : jj + 512], ptile[64:128, 0:512], AF.Exp,
                    accum_out=saccE[0:64, k : k + 1],
                )
                if k in (5, 7):
                    nc.scalar.copy(AfH[h][0:64, jj : jj + 512], ptile[0:64, 0:512])
                else:
                    nc.vector.tensor_copy(AfH[h][0:64, jj : jj + 512], ptile[0:64, 0:512])
                return k

            def vt_pair(kt, do_copy=True):
                pv = vtp.tile([128, 2 * C], f32, tag="pv", name=f"pv{kt}")
                for q in range(2):
                    nc.tensor.matmul(
                        pv[:, q * C : (q + 1) * C],
                        u8[:, :, (kt + q) * 128 : (kt + q + 1) * 128],
                        vw8[:, :, :],
                        start=True, stop=True, perf_mode=DR,
                    )
                if do_copy:
                    nc.scalar.copy(vtb[:, kt * C : (kt + 2) * C], pv[:, 0 : 2 * C])
                return pv

            # chunk order: j-halves interleaved so keys (cols 0:2048) and
            # chain inputs both complete early.  Chain half h's DVE work is
            # emitted as soon as its 4 chunks are in; the s matmuls are
            # emitted after ALL chunks so they never block chunk matmuls in
            # the in-order PE stream.
            def chain_stt(half):
                Wj = WjH[half]
                nc.vector.scalar_tensor_tensor(
                    Wj[64:128, :], Wj[0:64, :], 1.0,
                    AfH[half][0:64, :],
                    op0=ALU.mult, op1=ALU.mult,
                    accum_out=saccC[0:64, half : half + 1],
                )
                nc.vector.reduce_sum(
                    sigf[0:64, half : half + 1],
                    saccE[0:64, 4 * half : 4 * half + 4], axis=AX.X,
                )
                nc.vector.tensor_copy(
                    sigb[0:64, half : half + 1], sigf[0:64, half : half + 1]
                )
                nc.vector.tensor_copy(
                    sigb[64:128, half : half + 1], saccC[0:64, half : half + 1]
                )

            def s_mms(half):
                for kt in range(NKT):
                    nc.tensor.matmul(
                        s_ps[:, half * NKT + kt : half * NKT + kt + 1],
                        pwt[:, kt * 128 : (kt + 1) * 128],
                        sigb[:, half : half + 1],
                        start=True, stop=True,
                    )
                if half == 0:
                    nc.vector.tensor_copy(ssum[:], s_ps[:, 0:NKT])

            CHUNKS = (0, 1, 2, 3, 4, 5, 6, 7)
            for k, ci in enumerate(CHUNKS):
                ab_chunk(ci, k)
                if k == 3:
                    chain_stt(0)
            chain_stt(1)
            # vT runs in the chain window: PE + ACT are otherwise idle here.
            # The last two pair-copies go to DVE, interleaved into the pws
            # stream below, so ACT finishes in time for the moment phase.
            vt_late = {}
            for kt in range(0, NKT, 2):
                if kt < 12:
                    vt_pair(kt)
                else:
                    vt_late[kt] = vt_pair(kt, do_copy=False)
            s_mms(0)
            s_mms(1)
            nc.vector.tensor_tensor(
                ssum[:], ssum[:], s_ps[:, NKT : 2 * NKT], op=ALU.add
            )
            nc.vector.reciprocal(rinv[:], ssum[:])

            # ---- pws = pwsb * rinv (bf16, DVE 4x), then moment (bf16) ----
            for kt in range(NKT):
                if kt in vt_late:
                    nc.vector.tensor_copy(
                        vtb[:, kt * C : (kt + 2) * C], vt_late[kt][:, 0 : 2 * C]
                    )
                nc.vector.tensor_scalar(
                    pwsB[:, kt * BASIS : (kt + 1) * BASIS],
                    pwsb[:, kt * BASIS : (kt + 1) * BASIS],
                    rinv[:, kt : kt + 1], None, op0=ALU.mult,
                )
                nc.tensor.matmul(
                    mo_ps[:],
                    pwsB[:, kt * BASIS : (kt + 1) * BASIS],
                    vtb[:, kt * C : (kt + 1) * C],
                    start=(kt == 0), stop=(kt == NKT - 1),
                )
            nc.scalar.copy(mo0[:], mo_ps[:])

            # ---- P = Mo^T @ W -> fp8 out ----
            # psum ring alternates pa/pb tags (4-deep) so P matmuls never
            # stall on the copy drain; one contiguous 512KB DMA per c-block.
            for ct in range(2):
                for jg in range(8):
                    pp = abp.tile([128, 512], f32, tag="pa", name=f"pp{ct}_{jg}")
                    nc.tensor.matmul(
                        pp[:],
                        mo0[:, ct * 128 : (ct + 1) * 128],
                        WjH[jg // 4][:, (jg % 4) * 512 : (jg % 4 + 1) * 512],
                        start=True, stop=True,
                    )
                    dst = po[:, ct * HW + jg * 512 : ct * HW + (jg + 1) * 512]
                    if jg % 2 == 0:
                        nc.scalar.copy(dst, pp[:])
                    else:
                        nc.vector.tensor_copy(dst, pp[:])
                if ct == 0:
                    nc.gpsimd.dma_start(p_out[0:1, :, :], po[:, 0:HW])
                else:
                    nc.sync.dma_start(p_out[1:2, 0:48, :], po[0:48, HW : 2 * HW])
                    nc.scalar.dma_start(p_out[1:2, 48:96, :], po[48:96, HW : 2 * HW])
                    nc.gpsimd.dma_start(p_out[1:2, 96:128, :], po[96:128, HW : 2 * HW])

    nc.compile()
    return nc


def _get_built():
    global _BUILT
    if _BUILT is None:
        _BUILT = _build()
    return _BUILT


def _regions(kw1f, beta, t):
    """Region edges: ReLU breakpoints inside t-range, merged to <= RP-1,
    then wide regions split so max |t - center| shrinks (all slots used)."""
    tmin, tmax = t.min(), t.max()
    bp = -beta / np.where(np.abs(kw1f) < 1e-30, 1e-30, kw1f)
    inr = np.sort(bp[(bp > tmin) & (bp < tmax)])
    while len(inr) > RP - 1:
        gaps = np.diff(np.concatenate([[tmin], inr, [tmax]]))
        i = int(np.argmin(gaps[:-1] + gaps[1:]))
        inr = np.delete(inr, i)
    edges = list(inr)
    while len(edges) < RP - 1:
        full = np.concatenate([[tmin - 1e-9], edges, [tmax + 1e-9]])
        bi, bm, bsplit = -1, -1.0, None
        for i in range(len(full) - 1):
            selm = t[(t > full[i]) & (t <= full[i + 1])]
            if len(selm) < 2:
                continue
            c = 0.5 * (selm.min() + selm.max())
            m = np.abs(selm - c).max()
            if m > bm:
                bm, bi, bsplit = m, i, float(np.median(selm))
        if bi < 0:
            break
        edges.append(bsplit)
        edges.sort()
    return np.array(edges)


def _host_prep(boundary_map, uncertainty_map, key_w1, bn_scale, bn_bias,
               bn_mean, bn_var, key_w2, query_w, value_w):
    import ml_dtypes

    bf = ml_dtypes.bfloat16
    f8 = ml_dtypes.float8_e4m3
    b, c, h, w = uncertainty_map.shape
    H0 = boundary_map.shape[2]
    idx = (np.arange(h) * H0) // h
    bm = boundary_map[:, 0][:, idx][:, :, idx].reshape(b, h * w).astype(np.float64)

    inv = bn_scale.astype(np.float64) / np.sqrt(bn_var.astype(np.float64) + 1e-5)
    beta = bn_bias.astype(np.float64) - bn_mean.astype(np.float64) * inv
    kw1f = key_w1[:, 0].astype(np.float64) * inv
    m_t = key_w2.T.astype(np.float64) @ query_w.astype(np.float64)   # [64, 256]
    vw_t = np.ascontiguousarray(value_w.T.astype(np.float64))        # [256, 256]
    vw8 = vw_t.reshape(2, 128, C).transpose(1, 0, 2)                 # [128,2,256]

    in_maps = []
    for core in range(8):
        bi, kh = core // 2, core % 2
        t_full = bm[bi]
        tk = t_full[kh * KH : (kh + 1) * KH]
        u = uncertainty_map[bi].reshape(c, h * w).astype(np.float64)
        u = np.roll(u, -kh * KH, axis=1)
        u8 = u.reshape(2, 128, HW).transpose(1, 0, 2)                # [128,2,HW]

        edges = _regions(kw1f, beta, tk)
        R = len(edges) + 1
        reg = np.searchsorted(edges, tk)                             # [KH]
        lo = np.concatenate([[tk.min() - 1e-9], edges])
        hi = np.concatenate([edges, [tk.max() + 1e-9]])
        relu_mid = 0.5 * (lo + hi)
        masks = (kw1f[None, :] * relu_mid[:, None] + beta[None, :]) > 0  # [R,64]
        tc = np.zeros(R)
        hh = np.ones(R)
        for r_ in range(R):
            selk = tk[reg == r_]
            if len(selk):
                tc[r_] = 0.5 * (selk.min() + selk.max())
                hh[r_] = max(np.abs(selk - tc[r_]).max(), 1e-6)
        ca = masks * kw1f[None, :]                                   # [R,64]
        cb = masks * beta[None, :]
        camT = ((hh[:, None] * ca) @ m_t).T                          # [256, R]
        cbmT = ((cb + tc[:, None] * ca) @ m_t).T                     # [256, R]
        cabf = np.zeros((256, 2 * RP))
        cabf[:, 0:R] = camT
        cabf[:, RP : RP + R] = cbmT
        cab8 = cabf.reshape(2, 128, 2 * RP).transpose(1, 0, 2)       # [128,2,128]

        dlt = (tk - tc[reg]) / hh[reg]                               # [KH]
        pw = np.zeros((KH, BASIS))
        krange = np.arange(KH)
        pw[krange, reg] = 1.0
        pw[krange, RP + reg] = dlt
        # pwsb [128 kw, kt*BASIS+bc] = PSCALE * pw[kt*128+kw, bc]
        # (PSCALE folded back out on host via gamma)
        pwsb = (PSCALE * pw).reshape(NKT, 128, BASIS).transpose(1, 0, 2).reshape(
            128, NKT * BASIS)
        in_maps.append({
            "u8_in": np.ascontiguousarray(u8).astype(f8),
            "cab8_in": np.ascontiguousarray(cab8).astype(f8),
            "vw8_in": np.ascontiguousarray(vw8).astype(f8),
            "pwsb_in": np.ascontiguousarray(pwsb).astype(bf),
            "pwt_in": np.ascontiguousarray(pw.T).astype(bf),
        })
    return in_maps


def kernel(boundary_map, uncertainty_map, key_w1, bn_scale, bn_bias,
           bn_mean, bn_var, key_w2, query_w, value_w, gamma):
    global LAST_RESULTS
    from concourse.bass_utils import run_bass_kernel_spmd

    nc = _get_built()
    in_maps = _host_prep(
        np.asarray(boundary_map), np.asarray(uncertainty_map), np.asarray(key_w1),
        np.asarray(bn_scale), np.asarray(bn_bias), np.asarray(bn_mean),
        np.asarray(bn_var), np.asarray(key_w2), np.asarray(query_w),
        np.asarray(value_w),
    )
    kwargs = {}
    if TRACE:
        kwargs["trace"] = True
        if TRACE_CORES is not None:
            kwargs["trace_cores"] = TRACE_CORES
    res = run_bass_kernel_spmd(nc, in_maps, core_ids=list(range(8)), **kwargs)
    LAST_RESULTS = res

    b, c, h, w = uncertainty_map.shape
    g = np.float64(np.asarray(gamma).reshape(-1)[0]) / PSCALE
    out = np.empty((b, c, h * w), np.float32)
    um = np.asarray(uncertainty_map)
    for bi in range(b):
        P = (res.results[2 * bi]["p_out"].astype(np.float32).reshape(C, HW)
             + np.roll(res.results[2 * bi + 1]["p_out"].astype(np.float32).reshape(C, HW),
                       KH, axis=1))
        out[bi] = g * P + um[bi].reshape(c, h * w)
    return out.reshape(b, c, h, w)


# revision 40
# speedup vs baseline: 1.0732x; 1.0652x over previous
"""BoundaryAttentionModule Trainium2 kernel — centered moment expansion, fp8 DR.

Shapes (hardcoded): b=4, c=256, h=w=64 (HW=4096), mid=64, out_ch=256.
8 cores: core = (batch bi = core//2, key-half kh = core%2); each core
handles its 2048 keys against all 4096 queries j.

Math: E^T[k,j] = t_k*A_S[j] + B_S[j] within ReLU-region S of the scalar
boundary value t_k.  Expansion is CENTERED per region: with region
center t_S and half-width h_S, U[k,j] = exp(B'_S[j]) * exp(d A'_S[j])
where B' = B + t_S A, A' = h_S A, d = (t_k - t_S)/h_S in [-1,1].  The
host splits wide regions (64 region slots) so |d A'| is tiny and TWO
Taylor orders suffice: U ~ W0 + d*W1, W0 = exp(B'), W1 = W0*A'.
Host folds M = key_w2^T @ query_w into CA/CB: A'/B' come straight from
u via one fp8 DoubleRow matmul each (contraction c=256), no G2.

W is held as two per-j-half tiles Wj[h] [128=(n,S), 2048] (separate
tiles + separate accumulators give the Tile scheduler fine-grained
deps): rows 0:64 = W0 = exp(B') (ACT exp from psum, sigma0 via accum),
rows 64:128 = W1 = W0*A' (one scalar_tensor_tensor per half on DVE,
sigma1 via accum).  Per-half s matmuls start as soon as that half's
chain ends.  vT (fp8 DR, keys = j 0:2048) runs in the chain window;
pws = PSCALE*pw/s in bf16 (DVE 4x); moment + P in bf16; output fp8
written as two contiguous 512KB DMAs (host divides PSCALE via gamma).

HW notes baked in here: fp8 DoubleRow gives 2x contraction depth but
no column-rate gain; DVE fp8-dst elementwise ops are ~16x slow (keep
bf16); GPSIMD cannot touch PSUM; DMA throughput needs >=4KB-contiguous
runs on the HW-DGE queues (sync/scalar start fast, gpsimd aggregates
but starts ~4us late); the PE needs ~3us of continuous work to reach
2.4GHz (warm-up matmuls bridge the input-DMA window).
"""

import numpy as np

B, C, HW = 4, 256, 4096
KH = HW // 2          # 2048 keys per core
NKT = KH // 128       # 16 key tiles
RP = 64               # region slots
NORD = 2              # Taylor orders 0..1 (centered)
BASIS = NORD * RP     # 128
PSCALE = 128.0        # pws scale folded out on host via gamma

TRACE = False
TRACE_CORES = None
LAST_RESULTS = None

_BUILT = None


def _build():
    import concourse.bass as bass
    import concourse.tile as tile
    from concourse import bacc, mybir

    f32 = mybir.dt.float32
    bf16 = mybir.dt.bfloat16
    f8 = mybir.dt.float8e4
    AF = mybir.ActivationFunctionType
    AX = mybir.AxisListType
    ALU = mybir.AluOpType
    DR = mybir.MatmulPerfMode.DoubleRow

    nc = bacc.Bacc(
        "TRN2",
        target_bir_lowering=False,
        debug=False,
        enable_asserts=False,
        num_devices=8,
    )

    u8_in = nc.dram_tensor("u8_in", [128, 2, HW], f8, kind="ExternalInput").ap()
    cab8_in = nc.dram_tensor("cab8_in", [128, 2, 2 * RP], f8, kind="ExternalInput").ap()
    vw8_in = nc.dram_tensor("vw8_in", [128, 2, C], f8, kind="ExternalInput").ap()
    pwsb_in = nc.dram_tensor("pwsb_in", [128, NKT * BASIS], bf16, kind="ExternalInput").ap()
    pwt_in = nc.dram_tensor("pwt_in", [BASIS, KH], bf16, kind="ExternalInput").ap()
    p_out = nc.dram_tensor("p_out", [2, 128, HW], f8, kind="ExternalOutput").ap()

    with tile.TileContext(nc) as tc:
        with (
            tc.tile_pool(name="sb", bufs=1) as sb,
            tc.tile_pool(name="ab", bufs=4, space="PSUM") as abp,
            tc.tile_pool(name="wm", bufs=1, space="PSUM") as wmp,
            tc.tile_pool(name="vt", bufs=2, space="PSUM") as vtp,
            tc.tile_pool(name="pin", bufs=1, space="PSUM") as pinp,
        ):
            # ---- SBUF tiles ----
            u8 = sb.tile([128, 2, HW], f8, tag="u8", name="u8")
            cab8 = sb.tile([128, 2, 2 * RP], f8, tag="cab8", name="cab8")
            vw8 = sb.tile([128, 2, C], f8, tag="vw8", name="vw8")
            pwsb = sb.tile([128, NKT * BASIS], bf16, tag="pwsb", name="pwsb")
            pwsB = sb.tile([128, NKT * BASIS], bf16, tag="pwsB", name="pwsB")
            pwt = sb.tile([BASIS, KH], bf16, tag="pwt", name="pwt")
            Af0 = sb.tile([64, KH], bf16, tag="Af0", name="Af0")
            Af1 = sb.tile([64, KH], bf16, tag="Af1", name="Af1")
            Wj0 = sb.tile([128, KH], bf16, tag="Wj0", name="Wj0")
            Wj1 = sb.tile([128, KH], bf16, tag="Wj1", name="Wj1")
            AfH = (Af0, Af1)
            WjH = (Wj0, Wj1)
            vtb = sb.tile([128, NKT * C], bf16, tag="vtb", name="vtb")
            saccE = sb.tile([64, 8], f32, tag="saccE", name="saccE")
            saccC = sb.tile([64, 2], f32, tag="saccC", name="saccC")
            sigf = sb.tile([64, 2], f32, tag="sigf", name="sigf")
            sigb = sb.tile([128, 2], bf16, tag="sigb", name="sigb")
            rinv = sb.tile([128, NKT], f32, tag="rinv", name="rinv")
            mo0 = sb.tile([128, C], bf16, tag="mo0", name="mo0")
            po = sb.tile([128, 2 * HW], f8, tag="po", name="po")
            scr = sb.tile([128, 512], bf16, tag="scr", name="scr")
            nc.vector.memset(scr[:], 0.0)

            spin = pinp.tile([128, 512], f32, tag="spin", name="spin")
            s_ps = spin[:, 0 : 2 * NKT]
            mo_ps = spin[:, 256 : 256 + C]
            ssum = sb.tile([128, NKT], f32, tag="ssum", name="ssum")

            # ---- input DMAs ----
            # Only the HW-DGE queues (sync/scalar) start promptly; gpsimd's
            # SW-DGE adds ~4us. c-half u slices are 4KB runs -> fast packets.
            nc.gpsimd.dma_start(cab8[:], cab8_in[:, :, :])
            nc.gpsimd.dma_start(vw8[:], vw8_in[:, :, :])
            nc.sync.dma_start(u8[:, 0:1, 0:1024], u8_in[:, 0:1, 0:1024])
            nc.scalar.dma_start(u8[:, 1:2, 0:1024], u8_in[:, 1:2, 0:1024])
            nc.sync.dma_start(u8[:, 0:1, 1024:KH], u8_in[:, 0:1, 1024:KH])
            nc.scalar.dma_start(u8[:, 1:2, 1024:KH], u8_in[:, 1:2, 1024:KH])
            nc.gpsimd.dma_start(u8[:, 0:1, KH:HW], u8_in[:, 0:1, KH:HW])
            nc.sync.dma_start(u8[:, 1:2, KH:HW], u8_in[:, 1:2, KH:HW])
            nc.scalar.dma_start(pwt[:], pwt_in[:, :])
            nc.scalar.dma_start(pwsb[:], pwsb_in[:, :])

            # ---- PE warm-up while inputs stream ----
            def warm(i, n=1):
                for k in range(n):
                    pwm = wmp.tile([128, 512], f32, tag="wm", name=f"warm{i}_{k}")
                    nc.tensor.matmul(
                        pwm[:], scr[:, 0:128], scr[:, 0:512], start=True, stop=True
                    )

            warm("pre", 11)

            # ---- A|B matmul (one fp8 DR mm: out rows 0:64=A, 64:128=B),
            # exp from rows 64:128, Af copy from rows 0:64 ----
            def ab_chunk(ci, k):
                j0 = 512 * ci
                h, jj = ci // 4, 512 * (ci % 4)
                ptile = abp.tile([128, 512], f32, tag="pa", name=f"pab{ci}")
                nc.tensor.matmul(
                    ptile[:, 0:512], cab8[:, :, 0:128],
                    u8[:, :, j0 : j0 + 512],
                    start=True, stop=True, perf_mode=DR,
                )
                nc.scalar.activation(
                    WjH[h][0:64, jj : jj + 512], ptile[64:128, 0:512], AF.Exp,
                    accum_out=saccE[0:64, k : k + 1],
                )
                if k in (5, 7):
                    nc.scalar.copy(AfH[h][0:64, jj : jj + 512], ptile[0:64, 0:512])
                else:
                    nc.vector.tensor_copy(AfH[h][0:64, jj : jj + 512], ptile[0:64, 0:512])
                return k

            def vt_pair(kt, do_copy=True):
                pv = vtp.tile([128, 2 * C], f32, tag="pv", name=f"pv{kt}")
                for q in range(2):
                    nc.tensor.matmul(
                        pv[:, q * C : (q + 1) * C],
                        u8[:, :, (kt + q) * 128 : (kt + q + 1) * 128],
                        vw8[:, :, :],
                        start=True, stop=True, perf_mode=DR,
                    )
                if do_copy:
                    nc.scalar.copy(vtb[:, kt * C : (kt + 2) * C], pv[:, 0 : 2 * C])
                return pv

            # chunk order: j-halves interleaved so keys (cols 0:2048) and
            # chain inputs both complete early.  Chain half h's DVE work is
            # emitted as soon as its 4 chunks are in; the s matmuls are
            # emitted after ALL chunks so they never block chunk matmuls in
            # the in-order PE stream.
            def chain_stt(half):
                Wj = WjH[half]
                nc.vector.scalar_tensor_tensor(
                    Wj[64:128, :], Wj[0:64, :], 1.0,
                    AfH[half][0:64, :],
                    op0=ALU.mult, op1=ALU.mult,
                    accum_out=saccC[0:64, half : half + 1],
                )
                nc.vector.reduce_sum(
                    sigf[0:64, half : half + 1],
                    saccE[0:64, 4 * half : 4 * half + 4], axis=AX.X,
                )
                nc.vector.tensor_copy(
                    sigb[0:64, half : half + 1], sigf[0:64, half : half + 1]
                )
                nc.vector.tensor_copy(
                    sigb[64:128, half : half + 1], saccC[0:64, half : half + 1]
                )

            def s_mms(half):
                for kt in range(NKT):
                    nc.tensor.matmul(
                        s_ps[:, half * NKT + kt : half * NKT + kt + 1],
                        pwt[:, kt * 128 : (kt + 1) * 128],
                        sigb[:, half : half + 1],
                        start=True, stop=True,
                    )
                if half == 0:
                    nc.vector.tensor_copy(ssum[:], s_ps[:, 0:NKT])

            CHUNKS = (0, 1, 2, 3, 4, 5, 6, 7)
            for k, ci in enumerate(CHUNKS):
                ab_chunk(ci, k)
                if k == 3:
                    chain_stt(0)
            chain_stt(1)
            # vT runs in the chain window: PE + ACT are otherwise idle here.
            # The last two pair-copies go to DVE, interleaved into the pws
            # stream below, so ACT finishes in time for the moment phase.
            vt_late = {}
            for kt in range(0, NKT, 2):
                if kt < 12:
                    vt_pair(kt)
                else:
                    vt_late[kt] = vt_pair(kt, do_copy=False)
            s_mms(0)
            s_mms(1)
            nc.vector.tensor_tensor(
                ssum[:], ssum[:], s_ps[:, NKT : 2 * NKT], op=ALU.add
            )
            nc.vector.reciprocal(rinv[:], ssum[:])

            # ---- pws = pwsb * rinv (bf16, DVE 4x), then moment (bf16) ----
            for kt in range(NKT):
                if kt in vt_late:
                    nc.vector.tensor_copy(
                        vtb[:, kt * C : (kt + 2) * C], vt_late[kt][:, 0 : 2 * C]
                    )
                nc.vector.tensor_scalar(
                    pwsB[:, kt * BASIS : (kt + 1) * BASIS],
                    pwsb[:, kt * BASIS : (kt + 1) * BASIS],
                    rinv[:, kt : kt + 1], None, op0=ALU.mult,
                )
                nc.tensor.matmul(
                    mo_ps[:],
                    pwsB[:, kt * BASIS : (kt + 1) * BASIS],
                    vtb[:, kt * C : (kt + 1) * C],
                    start=(kt == 0), stop=(kt == NKT - 1),
                )
            nc.scalar.copy(mo0[:], mo_ps[:])

            # ---- P = Mo^T @ W -> fp8 out ----
            # psum ring alternates pa/pb tags (4-deep) so P matmuls never
            # stall on the copy drain; one contiguous 512KB DMA per c-block.
            for ct in range(2):
                for jg in range(8):
                    pp = abp.tile([128, 512], f32, tag="pa", name=f"pp{ct}_{jg}")
                    nc.tensor.matmul(
                        pp[:],
                        mo0[:, ct * 128 : (ct + 1) * 128],
                        WjH[jg // 4][:, (jg % 4) * 512 : (jg % 4 + 1) * 512],
                        start=True, stop=True,
                    )
                    dst = po[:, ct * HW + jg * 512 : ct * HW + (jg + 1) * 512]
                    if jg % 2 == 0:
                        nc.scalar.copy(dst, pp[:])
                    else:
                        nc.vector.tensor_copy(dst, pp[:])
                if ct == 0:
                    nc.gpsimd.dma_start(p_out[0:1, :, :], po[:, 0:HW])
                else:
                    nc.sync.dma_start(p_out[1:2, 0:48, :], po[0:48, HW : 2 * HW])
                    nc.scalar.dma_start(p_out[1:2, 48:96, :], po[48:96, HW : 2 * HW])
                    nc.gpsimd.dma_start(p_out[1:2, 96:128, :], po[96:128, HW : 2 * HW])

    nc.compile()
    return nc


def _get_built():
    global _BUILT
    if _BUILT is None:
        _BUILT = _build()
    return _BUILT


def _regions(kw1f, beta, t):
    """Region edges: ReLU breakpoints inside t-range, merged to <= RP-1,
    then wide regions split so max |t - center| shrinks (all slots used)."""
    tmin, tmax = t.min(), t.max()
    bp = -beta / np.where(np.abs(kw1f) < 1e-30, 1e-30, kw1f)
    inr = np.sort(bp[(bp > tmin) & (bp < tmax)])
    while len(inr) > RP - 1:
        gaps = np.diff(np.concatenate([[tmin], inr, [tmax]]))
        i = int(np.argmin(gaps[:-1] + gaps[1:]))
        inr = np.delete(inr, i)
    edges = list(inr)
    while len(edges) < RP - 1:
        full = np.concatenate([[tmin - 1e-9], edges, [tmax + 1e-9]])
        bi, bm, bsplit = -1, -1.0, None
        for i in range(len(full) - 1):
            selm = t[(t > full[i]) & (t <= full[i + 1])]
            if len(selm) < 2:
                continue
            c = 0.5 * (selm.min() + selm.max())
            m = np.abs(selm - c).max()
            if m > bm:
                bm, bi, bsplit = m, i, float(np.median(selm))
        if bi < 0:
            break
        edges.append(bsplit)
        edges.sort()
    return np.array(edges)


def _host_prep(boundary_map, uncertainty_map, key_w1, bn_scale, bn_bias,
               bn_mean, bn_var, key_w2, query_w, value_w):
    import ml_dtypes

    bf = ml_dtypes.bfloat16
    f8 = ml_dtypes.float8_e4m3
    b, c, h, w = uncertainty_map.shape
    H0 = boundary_map.shape[2]
    idx = (np.arange(h) * H0) // h
    bm = boundary_map[:, 0][:, idx][:, :, idx].reshape(b, h * w).astype(np.float64)

    inv = bn_scale.astype(np.float64) / np.sqrt(bn_var.astype(np.float64) + 1e-5)
    beta = bn_bias.astype(np.float64) - bn_mean.astype(np.float64) * inv
    kw1f = key_w1[:, 0].astype(np.float64) * inv
    m_t = key_w2.T.astype(np.float64) @ query_w.astype(np.float64)   # [64, 256]
    vw_t = np.ascontiguousarray(value_w.T.astype(np.float64))        # [256, 256]
    vw8 = vw_t.reshape(2, 128, C).transpose(1, 0, 2)                 # [128,2,256]

    in_maps = []
    for core in range(8):
        bi, kh = core // 2, core % 2
        t_full = bm[bi]
        tk = t_full[kh * KH : (kh + 1) * KH]
        u = uncertainty_map[bi].reshape(c, h * w).astype(np.float64)
        u = np.roll(u, -kh * KH, axis=1)
        u8 = u.reshape(2, 128, HW).transpose(1, 0, 2)                # [128,2,HW]

        edges = _regions(kw1f, beta, tk)
        R = len(edges) + 1
        reg = np.searchsorted(edges, tk)                             # [KH]
        lo = np.concatenate([[tk.min() - 1e-9], edges])
        hi = np.concatenate([edges, [tk.max() + 1e-9]])
        relu_mid = 0.5 * (lo + hi)
        masks = (kw1f[None, :] * relu_mid[:, None] + beta[None, :]) > 0  # [R,64]
        tc = np.zeros(R)
        hh = np.ones(R)
        for r_ in range(R):
            selk = tk[reg == r_]
            if len(selk):
                tc[r_] = 0.5 * (selk.min() + selk.max())
                hh[r_] = max(np.abs(selk - tc[r_]).max(), 1e-6)
        ca = masks * kw1f[None, :]                                   # [R,64]
        cb = masks * beta[None, :]
        camT = ((hh[:, None] * ca) @ m_t).T                          # [256, R]
        cbmT = ((cb + tc[:, None] * ca) @ m_t).T                     # [256, R]
        cabf = np.zeros((256, 2 * RP))
        cabf[:, 0:R] = camT
        cabf[:, RP : RP + R] = cbmT
        cab8 = cabf.reshape(2, 128, 2 * RP).transpose(1, 0, 2)       # [128,2,128]

        dlt = (tk - tc[reg]) / hh[reg]                               # [KH]
        pw = np.zeros((KH, BASIS))
        krange = np.arange(KH)
        pw[krange, reg] = 1.0
        pw[krange, RP + reg] = dlt
        # pwsb [128 kw, kt*BASIS+bc] = PSCALE * pw[kt*128+kw, bc]
        # (PSCALE folded back out on host via gamma)
        pwsb = (PSCALE * pw).reshape(NKT, 128, BASIS).transpose(1, 0, 2).reshape(
            128, NKT * BASIS)
        in_maps.append({
            "u8_in": np.ascontiguousarray(u8).astype(f8),
            "cab8_in": np.ascontiguousarray(cab8).astype(f8),
            "vw8_in": np.ascontiguousarray(vw8).astype(f8),
            "pwsb_in": np.ascontiguousarray(pwsb).astype(bf),
            "pwt_in": np.ascontiguousarray(pw.T).astype(bf),
        })
    return in_maps


def kernel(boundary_map, uncertainty_map, key_w1, bn_scale, bn_bias,
           bn_mean, bn_var, key_w2, query_w, value_w, gamma):
    global LAST_RESULTS
    from concourse.bass_utils import run_bass_kernel_spmd

    nc = _get_built()
    in_maps = _host_prep(
        np.asarray(boundary_map), np.asarray(uncertainty_map), np.asarray(key_w1),
        np.asarray(bn_scale), np.asarray(bn_bias), np.asarray(bn_mean),
        np.asarray(bn_var), np.asarray(key_w2), np.asarray(query_w),
        np.asarray(value_w),
    )
    kwargs = {}
    if TRACE:
        kwargs["trace"] = True
        if TRACE_CORES is not None:
            kwargs["trace_cores"] = TRACE_CORES
    res = run_bass_kernel_spmd(nc, in_maps, core_ids=list(range(8)), **kwargs)
    LAST_RESULTS = res

    b, c, h, w = uncertainty_map.shape
    g = np.float64(np.asarray(gamma).reshape(-1)[0]) / PSCALE
    out = np.empty((b, c, h * w), np.float32)
    um = np.asarray(uncertainty_map)
    for bi in range(b):
        P = (res.results[2 * bi]["p_out"].astype(np.float32).reshape(C, HW)
             + np.roll(res.results[2 * bi + 1]["p_out"].astype(np.float32).reshape(C, HW),
                       KH, axis=1))
        out[bi] = g * P + um[bi].reshape(c, h * w)
    return out.reshape(b, c, h, w)
